# revision 23
# baseline (speedup 1.0000x reference)
"""Ising-model energy kernel for 8 Trainium2 NeuronCores.

result = 0.25*S0 - 0.5*(Qup + Qdiag + S2)
  S0    = sum(A)                          (A = info_mtx)
  Qup   = sum over off-diagonal 128x128 tiles (t > g) of s_g^T A_tile s_t
  Qdiag = strict-upper part of the 64 diagonal tiles (host, fp64)
  S2    = sum_i A[i,i] s_i                (host, fp64)

Sharding: row-shard A into 8 slabs [1024, 8192], one per core, cast to fp8
e4m3 on host (tolerance is 2e-2; fp8 rounding on the big sums is O(100)
against an answer of ~8.4e6).

Measured constraints that shape the design:
- One sync-HWDGE stream pipelines packets gap-free at ~26.6GB/s per
  engine x 16 engines ~= 425GB/s for any row size >= 4KB, so chunk
  boundaries are free; but a kernel may use at most 8 HWDGE DMAs before
  the 8 completion semaphores recycle and the reuse-wait collides with
  a data-dep wait (a DMA can encode exactly one wait).  A second HWDGE
  queue adds no bandwidth, and gpsimd's SWDGE taxes DMA engine 79 (the
  queue-servicing engine) ~4us of ring polling — so: ONE queue, <= 8
  DMAs total, no SWDGE.
- The PE does a 512-column DoubleRow fp8 matmul in ~270-300ns and only
  consumes a chunk once it fully lands, so the input is streamed as 6
  chunks tapering to a single column-group: the PE tracks the stream
  and finishes ~1.1us after the last byte.
- Matmuls are ordered stationary-major within each chunk (p outermost)
  so consecutive matmuls share their LDWEIGHTS where possible.

PSUM accumulation commutes, so matmul order is free.  Column-groups
stream q-major (0..7 then 8..15); group 8+k's pair-3 matmul is PSUM
bank k's stop, so banks retire staggered over the second half of the
stream and the output path overlaps it.  Chunks:
  c0 [W | groups 0,1,2]  rows 512+12KB    c3 [groups 9,10,11]  12KB
  c1 [groups 3,4,5]      rows 12KB        c4 [groups 12,13,14] 12KB
  c2 [groups 6,7,8]      rows 12KB        c5 [group 15]        4KB

Each pair of 128-row blocks is the *moving* operand of DoubleRow fp8
matmuls (contraction 256 = 2 blocks x 128 rows) against a stationary
holding [s_block0 | s_block1 | ones] column triplets, so the PE consumes
two A elements per lane per cycle.  Column-group 8q+k accumulates into
the 16-row q-half of PSUM bank k.

Output is [32, 4096] fp16, all casts on vector in bank-retire order;
two output DMAs (banks 0-6, then bank 7 alone as a 32KB straggler) so
the final chain after the last matmul is one cast + one small DMA.
Within the 16-row half of column group 8q+k, rows 3p / 3p+1 are the
matvec u of blocks 2p / 2p+1 and row 3p+2 is the pair's column sum.
Host does the O(N)-sized mask/reduce and the exact diag-tile terms in
fp64.
"""

import numpy as np

N = 8192
NCORES = 8
ROWS = N // NCORES   # 1024 rows per core
BLK = 128            # partition block
NB = ROWS // BLK     # 8 row blocks per core
NPAIR = NB // 2      # 4 DoubleRow pairs per core
NT = N // BLK        # 64 column tiles (mask granularity)
GW = 512             # column-group width (one PSUM bank of fp32)
NG = N // GW         # 16 column groups
NBANK = 8            # PSUM banks used; 2 groups per bank
WTW = NPAIR * 2 * 64  # stationary width
# stream chunks: bank-interleaved column-groups (bank k owns groups k and
# 8+k; group 8+k's last matmul is bank k's stop), so PSUM banks retire
# staggered across the whole stream instead of bunching at the end.
# Within a chunk, same-q groups sit together so consecutive matmuls of a
# pair share their stationary.
CHUNK_COLS = [(0, 1, 8), (2, 9, 10), (3, 4, 11), (5, 12, 13), (6, 14), (7, 15)]

_NC_CACHE = None
LAST_EXEC_NS = None
LAST_RESULTS = None


def _build_nc():
    import concourse.bass as bass
    import concourse.tile as tile
    from concourse.tile_rust import add_dep_helper
    from concourse import mybir

    f32 = mybir.dt.float32
    f16 = mybir.dt.float16
    f8 = mybir.dt.float8e4
    dr = mybir.MatmulPerfMode.DoubleRow
    nc = bass.Bass()
    PHW = NPAIR * 2 * GW  # bytes per (column-group x all pairs) row slice
    dts = []
    for i, cols in enumerate(CHUNK_COLS):
        w = len(cols) * PHW + (WTW if i == 0 else 0)
        dts.append(nc.dram_tensor(f"c{i}", [BLK, w], f8, kind="ExternalInput"))
    o = nc.dram_tensor("o", [32, NBANK * GW], f16, kind="ExternalOutput")

    with tile.TileContext(nc) as tc:
        with (
            tc.tile_pool(name="data", bufs=1) as data,
            tc.tile_pool(name="psum", bufs=1, space="PSUM") as psum_pool,
        ):
            # Single sync HWDGE queue, issue order == consumption order.
            loads, views = [], []
            for i, (cols, dt) in enumerate(zip(CHUNK_COLS, dts)):
                t = data.tile(list(dt.shape), f8, tag=f"c{i}", name=f"c{i}")
                loads.append(nc.sync.dma_start(out=t, in_=dt[:, :]))
                body = t[:, WTW:] if i == 0 else t[:, :]
                views.append(
                    body.rearrange("r (c p h n) -> r c p h n", c=len(cols), p=NPAIR, h=2)
                )
                if i == 0:
                    w3 = t[:, :WTW].rearrange(
                        "r (s h m) -> r s h m", s=NPAIR * 2, h=2
                    )

            pbank = [
                psum_pool.tile([32, GW], f32, tag=f"pb{k}", name=f"pb{k}")
                for k in range(NBANK)
            ]

            mms = []
            for ci, cols in enumerate(CHUNK_COLS):
                for p in range(NPAIR):         # stationary-major: share LDW
                    for gi, g in enumerate(cols):
                        q, k = g // 8, g % 8
                        mms.append(
                            nc.tensor.matmul(
                                pbank[k][:, :],
                                w3[:, 2 * p + q, :, :],
                                views[ci][:, gi, p],
                                start=(q == 0 and p == 0),
                                stop=(q == 1 and p == NPAIR - 1),
                                perf_mode=dr,
                            )
                        )

            # All casts on vector in bank-retire order; two output DMAs on
            # sync (waits DVE>=6 / >=8, fresh sems: exactly 8 HWDGE DMAs).
            # Banks 0-5 retire during the stream, so their DMA queues right
            # behind the input and only banks 6-7 (64KB) trail the tail.
            obl = data.tile([32, 6 * GW], f16, tag="obl", name="obl")
            obh = data.tile([32, 2 * GW], f16, tag="obh", name="obh")
            cps = []
            for k in range(NBANK):
                dst = obl[:, GW * k : GW * (k + 1)] if k < 6 else \
                    obh[:, GW * (k - 6) : GW * (k - 5)]
                cps.append(nc.vector.tensor_copy(dst, pbank[k][:, :]))
            ods = [
                nc.sync.dma_start(out=o[:, : 6 * GW], in_=obl[:, :]),
                nc.sync.dma_start(out=o[:, 6 * GW :], in_=obh[:, :]),
            ]
            # The kernel-tail drain may carry only one sync wait; give SP a
            # 1-wait nop per otherwise-unobserved final semaphore tick so the
            # drain ends up with at most one wait left.  The scheduler may
            # reorder matmuls/casts, so absorb every candidate final tick.
            for dep in loads + mms + cps + ods:
                nop = nc.sync.nop()
                add_dep_helper(nop.ins, dep.ins, sync=True, reason="tail sem absorb")
    return nc


def _pack_inputs(A: np.ndarray, s: np.ndarray):
    import ml_dtypes

    f8 = ml_dtypes.float8_e4m3
    s_blocks = s.reshape(NT, BLK)  # s_blocks[g, i] = s[128*g + i]
    in_maps = []
    for d in range(NCORES):
        a8 = A[d * ROWS : (d + 1) * ROWS].astype(f8)
        # ap[p, h, r, g, col]: pair p holds blocks 2p (h=0) and 2p+1 (h=1)
        ap = a8.reshape(NPAIR, 2, BLK, NG, GW)
        W = np.zeros((BLK, WTW), dtype=f8)
        for p in range(NPAIR):
            s0 = s_blocks[d * NB + 2 * p].astype(f8)
            s1 = s_blocks[d * NB + 2 * p + 1].astype(f8)
            for q in range(2):
                base = 64 * (2 * p + q) + 16 * q + 3 * p
                W[:, base + 0] = s0        # h=0 slot of out row 16q+3p
                W[:, base + 32 + 1] = s1   # h=1 slot of out row 16q+3p+1
                W[:, base + 2] = 1.0       # colsum row gets both halves
                W[:, base + 32 + 2] = 1.0
        im = {}
        for i, cols in enumerate(CHUNK_COLS):
            # chunk row layout: [colgrp][pair][h][512]
            arr = ap[:, :, :, cols, :]            # [p, h, r, ci, col]
            arr = np.ascontiguousarray(arr.transpose(2, 3, 0, 1, 4)).reshape(
                BLK, len(cols) * NPAIR * 2 * GW
            )
            if i == 0:
                arr = np.concatenate([W, arr], axis=1)
            im[f"c{i}"] = np.ascontiguousarray(arr)
        in_maps.append(im)
    return in_maps


def kernel(info_mtx: np.ndarray, state: np.ndarray, _trace: bool = False) -> np.ndarray:
    global _NC_CACHE, LAST_EXEC_NS, LAST_RESULTS

    A = np.ascontiguousarray(np.asarray(info_mtx, dtype=np.float32))
    s = np.ascontiguousarray(np.asarray(state, dtype=np.float32))

    in_maps = _pack_inputs(A, s)

    if _NC_CACHE is None:
        _NC_CACHE = _build_nc()
    from concourse.bass_utils import run_bass_kernel_spmd

    res = run_bass_kernel_spmd(_NC_CACHE, in_maps, list(range(NCORES)), trace=_trace)
    LAST_EXEC_NS = res.exec_time_ns
    LAST_RESULTS = res

    s64 = s.astype(np.float64)
    # Decode: o[16q + 3p + r, 512k + off] covers column j = 512*(8q+k) + off;
    # r=0 -> u of block 2p, r=1 -> u of block 2p+1, r=2 -> pair column sum.
    U = np.empty((NCORES * NB, N), np.float64)
    S0 = 0.0
    urow_idx = [r for p in range(NPAIR) for r in (3 * p, 3 * p + 1)]
    for d in range(NCORES):
        oq = res.results[d]["o"].astype(np.float64).reshape(2, 16, NBANK, GW)
        U[d * NB : (d + 1) * NB] = (
            oq[:, urow_idx].transpose(1, 0, 2, 3).reshape(NB, N)
        )
        S0 += oq[:, 2::3].sum()

    # Mask at 128-column-tile granularity: block g contributes tiles t > g.
    per_tile = (U * s64[None, :]).reshape(NT, NT, BLK).sum(axis=2)
    Qup = np.triu(per_tile, k=1).sum()

    Qdiag = 0.0
    for g in range(NT):
        blk = A[g * BLK : (g + 1) * BLK, g * BLK : (g + 1) * BLK].astype(np.float64)
        sb = s64[g * BLK : (g + 1) * BLK]
        Qdiag += sb @ (np.triu(blk, 1) @ sb)
    S2 = float(np.diagonal(A).astype(np.float64) @ s64)

    result = 0.25 * S0 - 0.5 * (Qup + Qdiag + S2)
    return np.asarray(result, dtype=np.float32)
